# revision 5
# baseline (speedup 1.0000x reference)
"""HadamardMLPDecoder: 4-queue SWDGE dma_gather + src pair-sharing.

Within each core, edges with equal src are paired so two edges share one
src-row descriptor (phase A: superslots). Remaining edges are singles
(phase C). Groups are keyed by z-row windows (32768 rows, int16 indices);
-1 index tail-padding makes pad descriptors free. Compute: hadamard ->
PE transpose (bf16) -> W1 matmul -> relu -> W2 matmul. Host unpermutes.
"""

import numpy as np

import concourse.bass as bass
import concourse.mybir as mybir
import concourse.tile as tile
from concourse import bacc
from concourse.bass_utils import run_bass_kernel_spmd
from concourse.masks import make_identity
from contextlib import ExitStack

N, D, H = 100000, 128, 128
E_TOTAL = 2000000
NCORES = 8
P = 128
W = 32768
BLK = 4     # slots per compute block
CHUNK = 8   # slots (or superslots) per dma_gather chunk
E_CORE = E_TOTAL // NCORES

F32 = mybir.dt.float32
BF16 = mybir.dt.bfloat16
I16 = mybir.dt.int16
RELU = mybir.ActivationFunctionType.Relu
IDENT = mybir.ActivationFunctionType.Identity


def build_program(schedA, schedC, nsup, nsing):
    """schedA: (sup0, nsu, sw, wA, wB, nvS, nvA, nvB) chunks over superslots.
    schedC: (sl0, ns, sw, dw, nvS, nvD) chunks over single slots.
    idx tensor column sections (int16, wrapped 8 cols/slot):
      [srcA: nsup] [dstA: nsup] [dstB: nsup] [srcC: nsing] [dstC: nsing]
    out: [(2*nsup + nsing) * 128] f32; superslot w -> out slots 2w, 2w+1.
    """
    nc = bacc.Bacc("TRN2", target_bir_lowering=False, debug=False,
                   enable_asserts=False, num_devices=NCORES,
                   num_swdge_queues=4)
    ncols = 8 * (3 * nsup + 2 * nsing)
    z_d = nc.dram_tensor("z", [N, D], F32, kind="ExternalInput").ap()
    idx_d = nc.dram_tensor("idx", [P, ncols], I16, kind="ExternalInput").ap()
    w1_d = nc.dram_tensor("w1", [D, H], F32, kind="ExternalInput").ap()
    b1_d = nc.dram_tensor("b1", [H], F32, kind="ExternalInput").ap()
    w2_d = nc.dram_tensor("w2", [H, 1], F32, kind="ExternalInput").ap()
    b2_d = nc.dram_tensor("b2", [1], F32, kind="ExternalInput").ap()
    out_d = nc.dram_tensor("out", [(2 * nsup + nsing) * P], F32,
                           kind="ExternalOutput").ap()

    wins = [z_d[w * W : min((w + 1) * W, N), :] for w in range(4)]

    with tile.TileContext(nc) as tc, ExitStack() as ctx:
        const = ctx.enter_context(tc.tile_pool(name="const", bufs=1))
        zpool = ctx.enter_context(tc.tile_pool(name="gather", bufs=3))
        work = ctx.enter_context(tc.tile_pool(name="work", bufs=3))
        stage_pool = ctx.enter_context(tc.tile_pool(name="stage", bufs=3))
        psum_t = ctx.enter_context(tc.tile_pool(name="ps_t", bufs=2, space="PSUM"))
        psum_h = ctx.enter_context(tc.tile_pool(name="ps_h", bufs=2, space="PSUM"))
        psum_o = ctx.enter_context(tc.tile_pool(name="ps_o", bufs=2, space="PSUM"))

        idx_sb = const.tile([P, ncols], I16)
        nc.sync.dma_start(out=idx_sb[:], in_=idx_d[:, :])
        w1_sb = const.tile([P, H], F32)
        nc.sync.dma_start(out=w1_sb[:], in_=w1_d[:, :])
        b1_sb = const.tile([P, 1], F32)
        nc.sync.dma_start(out=b1_sb[:], in_=b1_d[:, None])
        w2_sb = const.tile([P, 1], F32)
        nc.sync.dma_start(out=w2_sb[:], in_=w2_d[:, :])
        b2_sb = const.tile([1, 1], F32)
        nc.sync.dma_start(out=b2_sb[:1], in_=b2_d[:, None])
        ident = const.tile([P, P], BF16)
        make_identity(nc, ident[:])
        w1_bf = const.tile([P, H], BF16)
        nc.vector.tensor_copy(out=w1_bf[:], in_=w1_sb[:])
        w2_bf = const.tile([P, 1], BF16)
        nc.vector.tensor_copy(out=w2_bf[:], in_=w2_sb[:])

        qn = [0]

        def gather(dst_ap, win, col0, cols, nidx, nvalid):
            nc.gpsimd.dma_gather(
                dst_ap, wins[win], idx_sb[:, col0 : col0 + cols],
                nidx, nvalid, D, queue_num=qn[0] % 4,
            )
            qn[0] += 1

        def mlp_block(ef, o_stage, e0):
            """ef: [P, BLK, D] bf16 tile -> outputs staged at o_stage[e0:]."""
            EB = BLK * P
            efT_ps = psum_t.tile([P, EB], BF16)
            for c in range(BLK):
                nc.tensor.transpose(
                    out=efT_ps[:, c * P : (c + 1) * P],
                    in_=ef[:, c, :], identity=ident[:],
                )
            efT = work.tile([P, EB], BF16, tag="efT")
            nc.vector.tensor_copy(out=efT[:], in_=efT_ps[:])
            h_ps = psum_h.tile([P, EB], F32)
            nc.tensor.matmul(out=h_ps[:], lhsT=w1_bf[:], rhs=efT[:],
                             start=True, stop=True)
            h_sb = work.tile([P, EB], BF16, tag="h")
            nc.scalar.activation(out=h_sb[:], in_=h_ps[:], func=RELU,
                                 bias=b1_sb[:, :1], scale=1.0)
            o_ps = psum_o.tile([1, EB], F32)
            nc.tensor.matmul(out=o_ps[:], lhsT=w2_bf[:], rhs=h_sb[:],
                             start=True, stop=True)
            nc.scalar.activation(
                out=o_stage[:1, e0 : e0 + EB], in_=o_ps[:], func=IDENT,
                bias=b2_sb[:1, :1], scale=1.0,
            )

        cS, cA, cB = 0, 8 * nsup, 16 * nsup
        # ---- phase A: superslots (one src cell serves two dst slots) ----
        for (u0, nu, sw, wA, wB, nvS, nvA, nvB) in schedA:
            zs = zpool.tile([P, CHUNK, D], F32, tag="zs")
            zda = zpool.tile([P, CHUNK, D], F32, tag="zda")
            zdb = zpool.tile([P, CHUNK, D], F32, tag="zdb")
            gather(zs[:, :nu, :], sw, cS + 8 * u0, 8 * nu, nu * P, nvS)
            gather(zda[:, :nu, :], wA, cA + 8 * u0, 8 * nu, nu * P, nvA)
            gather(zdb[:, :nu, :], wB, cB + 8 * u0, 8 * nu, nu * P, nvB)
            o_stage = stage_pool.tile([1, 2 * CHUNK * P], F32, tag="ostage")
            for b in range(nu // 2):  # block = 2 superslots -> 4 out slots
                w0 = b * 2
                ef = work.tile([P, BLK, D], BF16, tag="ef")
                nc.vector.tensor_mul(
                    out=ef[:, 0:2, :], in0=zs[:, w0 : w0 + 2, :],
                    in1=zda[:, w0 : w0 + 2, :])
                nc.vector.tensor_mul(
                    out=ef[:, 2:4, :], in0=zs[:, w0 : w0 + 2, :],
                    in1=zdb[:, w0 : w0 + 2, :])
                mlp_block(ef, o_stage, b * BLK * P)
            nc.sync.dma_start(
                out=out_d[(2 * u0) * P : (2 * u0 + 2 * nu) * P][None, :],
                in_=o_stage[:1, : 2 * nu * P],
            )

        # ---- phase C: single slots ----
        cSs, cDs = 24 * nsup, 24 * nsup + 8 * nsing
        obase = 2 * nsup
        for (s0, ns, sw, dw, nvS, nvD) in schedC:
            zs = zpool.tile([P, CHUNK, D], F32, tag="zs")
            zd = zpool.tile([P, CHUNK, D], F32, tag="zda")
            gather(zs[:, :ns, :], sw, cSs + 8 * s0, 8 * ns, ns * P, nvS)
            gather(zd[:, :ns, :], dw, cDs + 8 * s0, 8 * ns, ns * P, nvD)
            o_stage = stage_pool.tile([1, 2 * CHUNK * P], F32, tag="ostage")
            for b in range(ns // BLK):
                ef = work.tile([P, BLK, D], BF16, tag="ef")
                nc.vector.tensor_mul(out=ef[:, :, :],
                                     in0=zs[:, b * BLK : (b + 1) * BLK, :],
                                     in1=zd[:, b * BLK : (b + 1) * BLK, :])
                mlp_block(ef, o_stage, b * BLK * P)
            nc.sync.dma_start(
                out=out_d[(obase + s0) * P : (obase + s0 + ns) * P][None, :],
                in_=o_stage[:1, : ns * P],
            )

    nc.compile()
    return nc


def _wrap(flat):
    w = flat.reshape(-1, 16).T.astype(np.int16)
    return np.tile(w, (8, 1))


def pack_all(edge_label_index):
    src_f = np.asarray(edge_label_index[0], dtype=np.int64)
    dst_f = np.asarray(edge_label_index[1], dtype=np.int64)
    cores = []
    for c in range(NCORES):
        sl = slice(c * E_CORE, (c + 1) * E_CORE)
        s, d = src_f[sl], dst_f[sl]
        orig = np.arange(c * E_CORE, (c + 1) * E_CORE, dtype=np.int64)
        # greedy symmetric matching: the hadamard is symmetric in
        # (src, dst), so any two edges sharing ANY endpoint can share one
        # z-row descriptor (shared endpoint plays the "src" role).
        n = len(s)
        ev = np.concatenate([s, d])
        ee = np.concatenate([np.arange(n), np.arange(n)])
        o2 = np.argsort(ev, kind="stable")
        ev, ee = ev[o2], ee[o2]
        bnd = np.flatnonzero(np.diff(ev)) + 1
        buckets = np.split(ee, bnd)
        vals = ev[np.concatenate([[0], bnd])]
        used = np.zeros(n, bool)
        psl, dal, dbl, oal, obl = [], [], [], [], []
        for v, bk in zip(vals, buckets):
            prev = -1
            for e in bk:
                if used[e] or e == prev:
                    continue
                if prev < 0:
                    prev = e
                else:
                    used[prev] = used[e] = True
                    psl.append(v)
                    dal.append(d[prev] if s[prev] == v else s[prev])
                    dbl.append(d[e] if s[e] == v else s[e])
                    oal.append(orig[prev])
                    obl.append(orig[e])
                    prev = -1
        ps = np.array(psl, np.int64)
        da = np.array(dal, np.int64)
        db = np.array(dbl, np.int64)
        oa = np.array(oal, np.int64)
        ob = np.array(obl, np.int64)
        rem = ~used
        s, d, orig = s[rem], d[rem], orig[rem]
        # canonical window order
        swp = np.minimum(da // W, db // W)
        swq = np.maximum(da // W, db // W)
        flip = (da // W) > (db // W)
        da2 = np.where(flip, db, da)
        db2 = np.where(flip, da, db)
        oa2 = np.where(flip, ob, oa)
        ob2 = np.where(flip, oa, ob)
        gidA = (ps // W) * 16 + swp * 4 + swq
        gidC = (s // W) * 4 + d // W
        cores.append(((ps, da2, db2, oa2, ob2, gidA),
                      (s, d, orig, gidC)))
    # budgets per gid (max over cores), in superslots / slots
    gidsA = sorted(set(int(g) for core in cores for g in core[0][5]))
    gidsC = sorted(set(int(g) for core in cores for g in core[1][3]))
    budA = {g: max(-(-int((core[0][5] == g).sum()) // P) for core in cores)
            for g in gidsA}
    budC = {g: max(-(-int((core[1][3] == g).sum()) // P) for core in cores)
            for g in gidsC}
    # round up so every group is a multiple of 2 superslots / BLK slots
    for g in budA:
        budA[g] = -(-budA[g] // 2) * 2
    for g in budC:
        budC[g] = -(-budC[g] // BLK) * BLK
    nsup = sum(budA.values())
    nsing = sum(budC.values())

    # schedules (shared), with per-chunk valid counts = max over cores so the
    # ucode processes every core's valid descriptors (extra -1s are skipped
    # only if count says so -> use per-core counts? ucode asserts
    # num_idxs_reg == nonneg count, so counts must match per core exactly.
    # Simplest: make counts equal across cores by padding with window-base
    # index (0) instead of -1 up to the max-valid count, -1 beyond.
    schedA, schedC = [], []
    base = 0
    for g in gidsA:
        r = 0
        while r < budA[g]:
            nu = min(CHUNK, budA[g] - r)
            schedA.append([base + r, nu, g // 16, (g // 4) % 4, g % 4])
            r += nu
        base += budA[g]
    base = 0
    for g in gidsC:
        r = 0
        while r < budC[g]:
            ns = min(CHUNK, budC[g] - r)
            schedC.append([base + r, ns, g // 4, g % 4])
            r += ns
        base += budC[g]

    # per-core packing + per-chunk valid counts (must be uniform -> compute
    # per-core count per chunk, take max, and pad shorter cores with index 0
    # (valid, gathers window base row) so counts line up.
    nvalA = np.zeros((len(schedA), 3), np.int64)  # src, dstA, dstB
    nvalC = np.zeros((len(schedC), 2), np.int64)
    packedA, packedC = [], []
    for core in cores:
        (ps, da, db, oa, ob, gidA), (ss, sd, so, gidC) = core
        sA = np.full(nsup * P, -1, np.int64)
        dA = np.full(nsup * P, -1, np.int64)
        dB = np.full(nsup * P, -1, np.int64)
        oA = np.full(nsup * P, -1, np.int64)
        oB = np.full(nsup * P, -1, np.int64)
        basec = 0
        for g in gidsA:
            m = gidA == g
            k = int(m.sum())
            o0 = basec * P
            sA[o0 : o0 + k] = ps[m] - (g // 16) * W
            dA[o0 : o0 + k] = da[m] - ((g // 4) % 4) * W
            dB[o0 : o0 + k] = db[m] - (g % 4) * W
            oA[o0 : o0 + k] = oa[m]
            oB[o0 : o0 + k] = ob[m]
            basec += budA[g]
        sC = np.full(nsing * P, -1, np.int64)
        dC = np.full(nsing * P, -1, np.int64)
        oC = np.full(nsing * P, -1, np.int64)
        basec = 0
        for g in gidsC:
            m = gidC == g
            k = int(m.sum())
            o0 = basec * P
            sC[o0 : o0 + k] = ss[m] - (g // 4) * W
            dC[o0 : o0 + k] = sd[m] - (g % 4) * W
            oC[o0 : o0 + k] = so[m]
            basec += budC[g]
        packedA.append((sA, dA, dB, oA, oB))
        packedC.append((sC, dC, oC))
        for i, (u0, nu, *_rest) in enumerate(schedA):
            blk = slice(u0 * P, (u0 + nu) * P)
            nvalA[i, 0] = max(nvalA[i, 0], int((sA[blk] >= 0).sum()))
            nvalA[i, 1] = max(nvalA[i, 1], int((dA[blk] >= 0).sum()))
            nvalA[i, 2] = max(nvalA[i, 2], int((dB[blk] >= 0).sum()))
        for i, (s0, ns, *_rest) in enumerate(schedC):
            blk = slice(s0 * P, (s0 + ns) * P)
            nvalC[i, 0] = max(nvalC[i, 0], int((sC[blk] >= 0).sum()))
            nvalC[i, 1] = max(nvalC[i, 1], int((dC[blk] >= 0).sum()))

    # pad shorter cores to the max valid count with index 0
    out = []
    for (sA, dA, dB, oA, oB), (sC, dC, oC) in zip(packedA, packedC):
        for i, (u0, nu, *_r) in enumerate(schedA):
            for arr, col in ((sA, 0), (dA, 1), (dB, 2)):
                blk = arr[u0 * P : (u0 + nu) * P]
                need = int(nvalA[i, col]) - int((blk >= 0).sum())
                if need > 0:
                    fill = np.flatnonzero(blk < 0)[:need]
                    blk[fill] = 0
        for i, (s0, ns, *_r) in enumerate(schedC):
            for arr, col in ((sC, 0), (dC, 1)):
                blk = arr[s0 * P : (s0 + ns) * P]
                need = int(nvalC[i, col]) - int((blk >= 0).sum())
                if need > 0:
                    fill = np.flatnonzero(blk < 0)[:need]
                    blk[fill] = 0
        idx16 = np.concatenate(
            [_wrap(sA), _wrap(dA), _wrap(dB), _wrap(sC), _wrap(dC)], axis=1)
        # out-slot order per block b (2 superslots w0=2b, w0+1):
        # (A_w0, A_w0+1, B_w0, B_w0+1)
        oA2 = oA.reshape(nsup // 2, 2, P)
        oB2 = oB.reshape(nsup // 2, 2, P)
        origA = np.concatenate([oA2, oB2], axis=1).reshape(-1)
        out.append((np.ascontiguousarray(idx16),
                    np.concatenate([origA, oC])))

    schedA_t = tuple((u0, nu, sw, wa, wb, int(nvalA[i, 0]), int(nvalA[i, 1]),
                      int(nvalA[i, 2]))
                     for i, (u0, nu, sw, wa, wb) in enumerate(schedA))
    schedC_t = tuple((s0, ns, sw, dw, int(nvalC[i, 0]), int(nvalC[i, 1]))
                     for i, (s0, ns, sw, dw) in enumerate(schedC))
    return out, schedA_t, schedC_t, nsup, nsing


_NC_CACHE = {}


def run(inputs, trace=False, **kw):
    z = np.ascontiguousarray(np.asarray(inputs["z"], dtype=np.float32))
    w1 = np.ascontiguousarray(np.asarray(inputs["W1"], dtype=np.float32))
    b1v = np.ascontiguousarray(np.asarray(inputs["b1"], dtype=np.float32))
    w2 = np.ascontiguousarray(np.asarray(inputs["W2"], dtype=np.float32))
    b2v = np.ascontiguousarray(np.asarray(inputs["b2"], dtype=np.float32))
    packed, schedA, schedC, nsup, nsing = pack_all(inputs["edge_label_index"])
    key = (schedA, schedC, nsup, nsing)
    if key not in _NC_CACHE:
        _NC_CACHE[key] = build_program(schedA, schedC, nsup, nsing)
    res = run_bass_kernel_spmd(
        _NC_CACHE[key],
        [{"z": z, "idx": idx, "w1": w1, "b1": b1v, "w2": w2, "b2": b2v}
         for idx, _ in packed],
        list(range(NCORES)), trace=trace, **kw)
    outs = np.zeros(E_TOTAL, np.float32)
    for c in range(NCORES):
        dev = res.results[c]["out"]
        orig = packed[c][1]
        valid = orig >= 0
        outs[orig[valid]] = dev[valid]
    return outs, res


def kernel(z, edge_label_index, W1, b1, W2, b2):
    out, _ = run({"z": z, "edge_label_index": edge_label_index,
                  "W1": W1, "b1": b1, "W2": W2, "b2": b2})
    return out
